# revision 23
# baseline (speedup 1.0000x reference)
"""Trainium2 Bass kernel for a 3-layer hetero GraphSAGE encoder (gene/disease).

Strategy (8 NeuronCores, SPMD):
  - dst-nodes are bin-packed into (core, window-of-64) bins, balanced by degree;
    every edge lives with its dst node, so segment-mean is core-local.
  - Per window: indirect-DMA gather of src feature rows (fp32) from a DRAM
    table, one-hot matrices built on DVE, and a TensorE matmul G.T @ S that
    accumulates the segment MEAN (1/cnt folded into the one-hot) in PSUM,
    feature-major [d, 64].
  - Small matmuls apply Wl/Wr, bias via DVE (stats fused via accum_out).
  - BN stats AllReduce'd across the 8 cores; normalize + LeakyReLU + residual
    on a feature-major SBUF arena; PE transpose back to node-major; AllGather
    rebuilds the next layer's full gather table on every core.
"""

import os
import numpy as np

C = 8          # cores
W = 64         # dst window (nodes per bin)
P = 128        # partitions / edges per matmul sub-block
SLOPE = 0.01
EPS = 1e-5
PIECE_BYTES = 7 * 1024   # max gather-piece bytes per partition


# ----------------------------------------------------------------------------
# Host-side preprocessing
# ----------------------------------------------------------------------------

def _snake_pack(weights, n_bins, caps):
    """Assign items to bins (cap[b] items each) balancing sum(weights).

    Deals items in descending weight order, snaking across bins that still
    have capacity. Returns assign[item] = bin id.
    """
    n = len(weights)
    assert caps.sum() == n
    order = np.argsort(-weights, kind="stable")
    assign = np.empty(n, np.int64)
    used = np.zeros(n_bins, np.int64)
    pos = 0
    rnd = 0
    while pos < n:
        active = np.nonzero(used < caps)[0]
        if rnd % 2:
            active = active[::-1]
        take = min(len(active), n - pos)
        sel = active[:take]
        assign[order[pos:pos + take]] = sel
        used[sel] += 1
        pos += take
        rnd += 1
    return assign


MAX_GROUP_ROWS = 32768   # int16 index limit for dma_gather


def _pack_edge_meta(src_packed, dst_bin, dst_slot, scale, n_bins, n_src):
    """Organize edges per (bin, src-group); return per-group sorted arrays.

    Edges are grouped by src table group (each <= MAX_GROUP_ROWS rows so an
    int16 relative index addresses it). Returns (per_group, counts, grows):
    per_group[h] = (bin_id, rel_src i16, dst_slot, scale) sorted by bin;
    counts [n_bins, H].
    """
    H = max(1, -(-n_src // MAX_GROUP_ROWS))
    grows = -(-n_src // H)
    grp = src_packed // grows
    key = dst_bin * H + grp
    order = np.argsort(key, kind="stable")
    kk, ss, ssl, ssc = (key[order], src_packed[order], dst_slot[order],
                        scale[order])
    counts = np.bincount(kk, minlength=n_bins * H).reshape(n_bins, H)
    per_group = []
    for h in range(H):
        sel = (kk % H) == h
        per_group.append((kk[sel] // H, (ss[sel] - h * grows).astype(np.int16),
                          ssl[sel].astype(np.float32),
                          ssc[sel].astype(np.float32)))
    return per_group, counts, grows


def _window_meta(metas, nwin):
    """Build per-core meta [C, 128, total_cols] with per-window k's.

    metas: list over groups g of (per_group-tuple, counts [nbins, .]) entries
    flattened: each entry = (bin_id, rel, slot, sc, counts_col).
    Returns (arr, kws, offs): kws[g][w] sub-blocks for group g in window w
    (max over cores); offs[w] = column offset of window w's block.
    """
    nbins = nwin * C
    kws = []
    for (b, rel, slot, sc, cnt, gbase) in metas:
        grid = cnt.reshape(C, nwin)
        kws.append(np.maximum(1, -(-grid.max(axis=0) // P)))
    offs = np.zeros(nwin + 1, np.int64)
    for w in range(nwin):
        offs[w + 1] = offs[w] + 7 * sum(int(k[w]) for k in kws)
    arr = np.zeros((C, P, int(offs[-1])), np.int32)
    for g, (b, rel, slot, sc, cnt, gbase) in enumerate(metas):
        # position of each edge within its (bin, group)
        o = np.zeros(nbins + 1, np.int64)
        np.cumsum(cnt.reshape(-1), out=o[1:])
        pos = np.arange(len(b)) - o[b]
        core = b // nwin
        w = b % nwin
        for ww in range(nwin):
            base = offs[ww] + 7 * sum(int(k[ww]) for k in kws[:g])
            sel = w == ww
            if not sel.any():
                continue
            kk = int(kws[g][ww])
            i = pos[sel]
            cc = core[sel]
            idx16 = np.zeros((C, 16, kk * 8), np.int16)
            idx16[cc, i % 16, i // 16] = rel[sel]
            idx16 = np.tile(idx16, (1, 8, 1)).view(np.int32)
            arr[:, :, base:base + 4 * kk] = idx16
            # absolute int32 indices, p-major (for indirect_dma_start)
            iv = np.zeros((C, P, kk), np.int32)
            iv[cc, i % P, i // P] = (rel[sel].astype(np.int32) + gbase)
            arr[:, :, base + 4 * kk:base + 5 * kk] = iv
            dv = np.zeros((C, P, kk), np.float32)
            scv = np.zeros((C, P, kk), np.float32)
            dv[cc, i % P, i // P] = slot[sel]
            scv[cc, i % P, i // P] = sc[sel]
            arr[:, :, base + 5 * kk:base + 6 * kk] = dv.view(np.int32)
            arr[:, :, base + 6 * kk:base + 7 * kk] = scv.view(np.int32)
    return arr, kws, offs


def _prep(x_gene, x_dis, e_gg, e_gd, e_dg, params):
    NG, DIN = x_gene.shape
    ND = x_dis.shape[0]
    D = 64
    assert NG % C == 0 and ND % C == 0
    npc_g, npc_d = NG // C, ND // C                  # real nodes per core
    nwin_g, nwin_d = -(-npc_g // W), -(-npc_d // W)  # windows per core
    last_g = npc_g - (nwin_g - 1) * W                # real cols in last window
    last_d = npc_d - (nwin_d - 1) * W
    NGP, NDP = C * nwin_g * W, C * nwin_d * W        # padded table sizes

    e_gg = np.asarray(e_gg, np.int64)
    e_gd = np.asarray(e_gd, np.int64)
    e_dg = np.asarray(e_dg, np.int64)

    deg_gg = np.bincount(e_gg[1], minlength=NG).astype(np.float64)
    deg_dg = np.bincount(e_dg[1], minlength=NG).astype(np.float64)
    deg_gd = np.bincount(e_gd[1], minlength=ND).astype(np.float64)

    # ---- bin-pack gene nodes into C*nwin_g bins (cap W, last window smaller)
    nbins_g = C * nwin_g
    caps_g = np.full(nbins_g, W, np.int64)
    caps_g[np.arange(C) * nwin_g + (nwin_g - 1)] = last_g
    wsum_g = deg_gg / max(deg_gg.mean(), 1e-9) + deg_dg / max(deg_dg.mean(), 1e-9)
    bin_g = _snake_pack(wsum_g, nbins_g, caps_g)

    nbins_d = C * nwin_d
    caps_d = np.full(nbins_d, W, np.int64)
    caps_d[np.arange(C) * nwin_d + (nwin_d - 1)] = last_d
    bin_d = _snake_pack(deg_gd.astype(np.float64), nbins_d, caps_d)

    # slot within bin, packed id
    def slots(bin_assign, n_bins):
        order = np.argsort(bin_assign, kind="stable")
        counts = np.bincount(bin_assign, minlength=n_bins)
        offs = np.zeros(n_bins + 1, np.int64)
        np.cumsum(counts, out=offs[1:])
        slot = np.empty_like(bin_assign)
        slot[order] = np.arange(len(bin_assign)) - offs[bin_assign[order]]
        return slot

    slot_g = slots(bin_g, nbins_g)
    slot_d = slots(bin_d, nbins_d)
    gpack = bin_g * W + slot_g      # packed id in [0, NGP)
    dpack = bin_d * W + slot_d

    # ---- edge meta per type --------------------------------------------------
    scale_gg = (1.0 / np.maximum(deg_gg, 1.0))[e_gg[1]]
    scale_dg = (1.0 / np.maximum(deg_dg, 1.0))[e_dg[1]]
    scale_gd = (1.0 / np.maximum(deg_gd, 1.0))[e_gd[1]]

    def meta(e, scale, src_pack, dst_bin_assign, dst_slot, n_bins, n_src):
        sbin = dst_bin_assign[e[1]]
        sslot = dst_slot[e[1]]
        return _pack_edge_meta(src_pack[e[0]], sbin, sslot, scale, n_bins,
                               n_src)

    m_gg, c_gg, grows_g = meta(e_gg, scale_gg, gpack, bin_g, slot_g, nbins_g,
                               NGP)
    m_dg, c_dg, grows_d = meta(e_dg, scale_dg, dpack, bin_g, slot_g, nbins_g,
                               NDP)
    m_gd, c_gd, _ = meta(e_gd, scale_gd, gpack, bin_d, slot_d, nbins_d, NGP)

    gentries = ([m + (c_gg[:, h], h * grows_g) for h, m in enumerate(m_gg)]
                + [m + (c_dg[:, h], h * grows_d) for h, m in enumerate(m_dg)])
    dentries = [m + (c_gd[:, h], h * grows_g) for h, m in enumerate(m_gd)]
    gmeta, gkws, goffs = _window_meta(gentries, nwin_g)
    dmeta, dkws, doffs = _window_meta(dentries, nwin_d)
    n_gg_groups = len(m_gg)

    # ---- packed feature tables / transposed shards --------------------------
    xg0 = np.zeros((NGP, DIN), np.float32)
    xg0[gpack] = np.asarray(x_gene, np.float32)
    xd0 = np.zeros((NDP, DIN), np.float32)
    xd0[dpack] = np.asarray(x_dis, np.float32)
    xgt0 = np.ascontiguousarray(
        xg0.reshape(C, nwin_g * W, DIN).transpose(0, 2, 1))  # [C, DIN, nwin*W]
    xdt0 = np.ascontiguousarray(
        xd0.reshape(C, nwin_d * W, DIN).transpose(0, 2, 1))

    # ---- weights wall [128, NW] ---------------------------------------------
    cols = []

    def f32(a):
        return np.asarray(a, np.float32)

    def padP(a):   # pad [F, c] to [128, c]
        a = f32(a)
        out = np.zeros((P, a.shape[1]), np.float32)
        out[: a.shape[0]] = a
        return out

    layout = {}

    def add(name, arr):
        layout[name] = (sum(c.shape[1] for c in cols), arr.shape[1])
        cols.append(padP(arr))

    add("I", np.eye(P, dtype=np.float32))
    add("J", np.tile(np.arange(W, dtype=np.float32)[None, :], (P, 1)))
    L = params["layers"]
    for li in range(3):
        p = L[li]
        add(f"wl_gg{li}", 0.5 * f32(p["gg"]["Wl"]))
        add(f"wl_dg{li}", 0.5 * f32(p["dg"]["Wl"]))
        add(f"wl_gd{li}", f32(p["gd"]["Wl"]))
        add(f"wr_g{li}", 0.5 * (f32(p["gg"]["Wr"]) + f32(p["dg"]["Wr"])))
        add(f"wr_d{li}", f32(p["gd"]["Wr"]))
        add(f"bg{li}", (0.5 * (f32(p["gg"]["b"]) + f32(p["dg"]["b"])))[:, None])
        add(f"bd{li}", f32(p["gd"]["b"])[:, None])
        add(f"gam_g{li}", f32(p["bn_gene"]["g"])[:, None])
        add(f"bet_g{li}", f32(p["bn_gene"]["b"])[:, None])
        add(f"gam_d{li}", f32(p["bn_dis"]["g"])[:, None])
        add(f"bet_d{li}", f32(p["bn_dis"]["b"])[:, None])
    for nt in ("gene", "dis"):
        q = params["post"][nt]
        s = "g" if nt == "gene" else "d"
        add(f"w1{s}", f32(q["lin1"]["W"]))
        add(f"w2{s}", f32(q["lin2"]["W"]))
        add(f"b1{s}", f32(q["lin1"]["b"])[:, None])
        add(f"b2{s}", f32(q["lin2"]["b"])[:, None])
        add(f"gam_p{s}", f32(q["bn"]["g"])[:, None])
        add(f"bet_p{s}", f32(q["bn"]["b"])[:, None])
    wall = np.concatenate(cols, axis=1)

    cfg = dict(
        NG=NG, ND=ND, DIN=DIN, D=D, NGP=NGP, NDP=NDP,
        npc_g=npc_g, npc_d=npc_d, nwin_g=nwin_g, nwin_d=nwin_d,
        last_g=last_g, last_d=last_d,
        gkws=[k.tolist() for k in gkws], dkws=[k.tolist() for k in dkws],
        goffs=goffs.tolist(), doffs=doffs.tolist(),
        n_gg_groups=n_gg_groups,
        grows_g=grows_g, grows_d=grows_d,
        wall_cols=wall.shape[1], layout=layout,
    )
    host = dict(xg0=xg0, xd0=xd0, xgt0=xgt0, xdt0=xdt0,
                gmeta=gmeta, dmeta=dmeta, wall=wall,
                gpack=gpack, dpack=dpack)
    return cfg, host


# ----------------------------------------------------------------------------
# Device program
# ----------------------------------------------------------------------------

def _pieces(k, F):
    npc = max(1, -(-(k * F * 4) // PIECE_BYTES))
    npc = min(npc, k)
    bounds = np.linspace(0, k, npc + 1).astype(int)
    return [(int(a), int(b)) for a, b in zip(bounds[:-1], bounds[1:]) if b > a]


def _build(nc, tc, cfg, T):
    import concourse.bass as bass
    import concourse.mybir as mybir
    from contextlib import ExitStack

    f32 = mybir.dt.float32
    i32 = mybir.dt.int32
    i16 = mybir.dt.int16
    OP = mybir.AluOpType
    AF = mybir.ActivationFunctionType

    D = cfg["D"]
    DIN = cfg["DIN"]
    nwin_g, nwin_d = cfg["nwin_g"], cfg["nwin_d"]
    gkws, dkws = cfg["gkws"], cfg["dkws"]
    goffs, doffs = cfg["goffs"], cfg["doffs"]
    ngg = cfg["n_gg_groups"]
    Bg = max(goffs[w + 1] - goffs[w] for w in range(nwin_g))
    Bd = max(doffs[w + 1] - doffs[w] for w in range(nwin_d))
    AG = nwin_g * W     # arena cols gene
    AD = nwin_d * W
    lay = cfg["layout"]

    ctx = ExitStack()
    with ctx:
        consts = ctx.enter_context(tc.tile_pool(name="consts", bufs=1))
        arenas = ctx.enter_context(tc.tile_pool(name="arenas", bufs=1))
        meta_p = ctx.enter_context(tc.tile_pool(name="meta", bufs=3))
        gat_p = ctx.enter_context(tc.tile_pool(
            name="gather", bufs=int(os.environ.get("BASSGNN_GBUFS", "2"))))
        s_p = ctx.enter_context(tc.tile_pool(
            name="onehot", bufs=int(os.environ.get("BASSGNN_SBUFS", "2"))))
        sm_p = ctx.enter_context(tc.tile_pool(name="small", bufs=2))
        st_p = ctx.enter_context(tc.tile_pool(name="stats", bufs=1))
        ps_p = ctx.enter_context(tc.tile_pool(name="psum", bufs=2, space="PSUM"))
        dram = ctx.enter_context(tc.tile_pool(name="dram", bufs=1, space="DRAM"))

        wall = consts.tile([P, cfg["wall_cols"]], f32)
        nc.sync.dma_start(wall[:], T["wall"][:, :])

        def wv(name):
            o, n = lay[name]
            return wall[:, o:o + n]

        def wv64(name, F=D):
            o, n = lay[name]
            return wall[:F, o:o + n]

        I64 = wall[:D, lay["I"][0]:lay["I"][0] + D]
        J = wv("J")  # [128, W] f32 iota row

        # persistent arenas (feature-major)
        gA = arenas.tile([P, AG], f32, tag="gA")
        gB = arenas.tile([P, AG], f32, tag="gB")
        dA = arenas.tile([P, AD], f32, tag="dA")
        dB = arenas.tile([P, AD], f32, tag="dB")
        mean_g = arenas.tile([P, AG], f32, tag="mean")

        # next-layer gather tables (AllGather outputs, Shared)
        xg_t = [T["xg0"]]
        xd_t = [T["xd0"]]
        for li in (1, 2):
            xg_t.append(dram.tile([cfg["NGP"], D], f32, tag=f"xg{li}",
                                  name=f"xgtab{li}", addr_space="Shared")[:])
            xd_t.append(dram.tile([cfg["NDP"], D], f32, tag=f"xd{li}",
                                  name=f"xdtab{li}", addr_space="Shared")[:])

        rg = [list(range(C))]

        def sweep(table, grows, F, meta_tile, groups, dst_psum_fn, typ):
            """Aggregation for one window: per src-group gather+one-hot+matmul.

            groups: list of (k_subblocks, meta col offset, src group h).
            """
            agg = dst_psum_fn()
            ktot = sum(k for k, _, _ in groups)
            jglob = 0
            for (k, off, h) in groups:
                idx16_ap = meta_tile[:, off:off + 4 * k].bitcast(i16)
                idx32_ap = meta_tile[:, off + 4 * k:off + 5 * k]
                dstv_ap = meta_tile[:, off + 5 * k:off + 6 * k].bitcast(f32)
                sc_ap = meta_tile[:, off + 6 * k:off + 7 * k].bitcast(f32)
                rows = min(grows, table.shape[0] - h * grows)
                G = gat_p.tile([P, k * F], f32, tag=f"g_{typ}",
                               name=f"g_{typ}")
                if F == 64:
                    # one-idx-per-partition indirect gather: 25ns/row for
                    # 256B rows (vs 38 for dma_gather)
                    for j in range(k):
                        nc.gpsimd.indirect_dma_start(
                            out=G[:, j * F:(j + 1) * F],
                            out_offset=None,
                            in_=table,
                            in_offset=bass.IndirectOffsetOnAxis(
                                ap=idx32_ap[:, j:j + 1], axis=0),
                        )
                else:
                    nc.gpsimd.dma_gather(
                        out_ap=G[:].rearrange("p (k f) -> p k f", f=F),
                        in_ap=table[h * grows:h * grows + rows, :],
                        idxs_ap=idx16_ap,
                        num_idxs=k * P,
                        num_idxs_reg=k * P,
                        elem_size=F,
                        single_packet=False,
                    )
                S = s_p.tile([P, k * W], f32, tag=f"s_{typ}", name=f"s_{typ}")
                S3 = S[:].rearrange("p (k w) -> p k w", w=W)
                dv3 = dstv_ap.unsqueeze(2).broadcast_to([P, k, W])
                J3 = J.unsqueeze(1).broadcast_to([P, k, W])
                nc.vector.tensor_tensor(out=S3, in0=dv3, in1=J3, op=OP.is_equal)
                sc3 = sc_ap.unsqueeze(2).broadcast_to([P, k, W])
                nc.vector.tensor_tensor(out=S3, in0=S3, in1=sc3, op=OP.mult)
                for j in range(k):
                    nc.tensor.matmul(
                        agg[:],
                        lhsT=G[:, j * F:(j + 1) * F],
                        rhs=S[:, j * W:(j + 1) * W],
                        start=(jglob == 0), stop=(jglob == ktot - 1),
                    )
                    jglob += 1
            return agg

        def wgroups(kws, w, g0, g1):
            out = []
            off = 7 * sum(int(kws[g][w]) for g in range(g0))
            for g in range(g0, g1):
                k = int(kws[g][w])
                out.append((k, off, g - g0))
                off += 7 * k
            return out

        def stats_cols(arena, ws, w, bias_col, psum, ssum, ssq, ncols):
            """psum [D, W] + bias -> arena[:, ws]; accumulate sum/sumsq."""
            if ncols == W:
                nc.vector.tensor_scalar(
                    out=arena[:D, ws], in0=psum[:], scalar1=bias_col, scalar2=None,
                    op0=OP.add, op1=OP.add, accum_out=ssum[:, w:w + 1])
                sq = sm_p.tile([D, W], f32, tag="sq")
                nc.vector.scalar_tensor_tensor(
                    out=sq[:], in0=arena[:D, ws], scalar=1.0, in1=arena[:D, ws],
                    op0=OP.mult, op1=OP.mult, accum_out=ssq[:, w:w + 1])
            else:
                nc.vector.tensor_scalar(
                    out=arena[:D, ws], in0=psum[:], scalar1=bias_col, scalar2=None,
                    op0=OP.add)
                sl = arena[:D, ws.start:ws.start + ncols]
                part = sm_p.tile([D, W], f32, tag="sq")
                nc.vector.tensor_scalar(
                    out=part[:, :ncols], in0=sl, scalar1=0.0, scalar2=None,
                    op0=OP.add, op1=OP.add, accum_out=ssum[:, w:w + 1])
                nc.vector.scalar_tensor_tensor(
                    out=part[:, :ncols], in0=sl, scalar=1.0, in1=sl,
                    op0=OP.mult, op1=OP.mult, accum_out=ssq[:, w:w + 1])

        def bn_normalize(arena, ncols, ssum, ssq, nwin, n_real, gam, bet,
                         resid_arena, out_rows=D):
            """AllReduce stats; arena = lrelu(bn(arena)) (+ resid)."""
            st2 = sm_p.tile([D, 2], f32, tag="st2")
            nc.vector.tensor_reduce(out=st2[:, 0:1], in_=ssum[:],
                                    op=OP.add, axis=mybir.AxisListType.X)
            nc.vector.tensor_reduce(out=st2[:, 1:2], in_=ssq[:],
                                    op=OP.add, axis=mybir.AxisListType.X)
            cin = dram.tile([D, 2], f32, tag="cc_in")
            cout = dram.tile([D, 2], f32, tag="cc_out", addr_space="Shared")
            nc.sync.dma_start(cin[:], st2[:])
            nc.gpsimd.collective_compute(
                "AllReduce", OP.add, replica_groups=rg,
                ins=[cin[:].opt()], outs=[cout[:].opt()])
            rstat = sm_p.tile([D, 2], f32, tag="rstat")
            nc.sync.dma_start(rstat[:], cout[:])
            mcol = sm_p.tile([D, 1], f32, tag="mcol")
            vcol = sm_p.tile([D, 1], f32, tag="vcol")
            acol = sm_p.tile([D, 1], f32, tag="acol")
            bcol = sm_p.tile([D, 1], f32, tag="bcol")
            inv_n = 1.0 / float(n_real)
            nc.vector.tensor_scalar(out=mcol[:], in0=rstat[:, 0:1],
                                    scalar1=inv_n, scalar2=None, op0=OP.mult)
            # v = E[x^2] - m^2 + eps
            nc.vector.tensor_scalar(out=vcol[:], in0=rstat[:, 1:2],
                                    scalar1=inv_n, scalar2=None, op0=OP.mult)
            m2 = sm_p.tile([D, 1], f32, tag="m2")
            nc.vector.tensor_tensor(out=m2[:], in0=mcol[:], in1=mcol[:],
                                    op=OP.mult)
            nc.vector.tensor_tensor(out=vcol[:], in0=vcol[:], in1=m2[:],
                                    op=OP.subtract)
            nc.vector.tensor_scalar(out=vcol[:], in0=vcol[:], scalar1=EPS,
                                    scalar2=None, op0=OP.add)
            if "dbg_rstat" in T:
                i = T["_dbg_i"] = T.get("_dbg_i", -1) + 1
                nc.sync.dma_start(out=T["dbg_rstat"][:, 2 * i:2 * i + 2],
                                  in_=rstat[:])
                nc.sync.dma_start(out=T["dbg_var"][:, i:i + 1], in_=vcol[:])
            if os.environ.get("BASSGNN_NOSQRT") == "1":
                nc.vector.tensor_scalar(out=vcol[:], in0=vcol[:], scalar1=1.0,
                                        scalar2=None, op0=OP.max)
            else:
                nc.scalar.sqrt(out=vcol[:], in_=vcol[:])
            nc.vector.reciprocal(out=acol[:], in_=vcol[:])
            nc.vector.tensor_tensor(out=acol[:], in0=acol[:], in1=gam[:D, :],
                                    op=OP.mult)
            nc.vector.tensor_tensor(out=bcol[:], in0=mcol[:], in1=acol[:],
                                    op=OP.mult)
            nc.vector.tensor_tensor(out=bcol[:], in0=bet[:D, :], in1=bcol[:],
                                    op=OP.subtract)
            sl = arena[:out_rows, :ncols]
            nc.vector.tensor_scalar(out=sl, in0=sl, scalar1=acol[:],
                                    scalar2=bcol[:], op0=OP.mult, op1=OP.add)
            nc.vector.scalar_tensor_tensor(out=sl, in0=sl, scalar=SLOPE,
                                           in1=sl, op0=OP.mult, op1=OP.max)
            if resid_arena is not None:
                nc.vector.tensor_tensor(out=sl, in0=sl,
                                        in1=resid_arena[:out_rows, :ncols],
                                        op=OP.add)

        def transpose_out(arena, rows0, ncols, dst_dram):
            """arena[rows0:rows0+64, :ncols] -> dst_dram [ncols, 64] node-major."""
            nch = -(-ncols // P)
            for c in range(nch):
                cs = min(P, ncols - c * P)
                tp = ps_p.tile([P, D], f32, tag="tp")
                nc.tensor.transpose(
                    out=tp[:cs, :],
                    in_=arena[rows0:rows0 + D, c * P:c * P + cs],
                    identity=I64)
                stg = sm_p.tile([P, D], f32, tag="tstage")
                nc.vector.tensor_copy(out=stg[:cs, :], in_=tp[:cs, :])
                nc.sync.dma_start(out=dst_dram[c * P:c * P + cs, :],
                                  in_=stg[:cs, :])

        # ------------------------------------------------------------------
        cur_g, cur_d = None, None          # feature-major arenas of layer input
        for li in range(3):
            F = DIN if li == 0 else D
            og = [gA, gB, gA][li]
            od = [dA, dB, dA][li]
            gsum = st_p.tile([D, nwin_g], f32, tag="gsum")
            gsq = st_p.tile([D, nwin_g], f32, tag="gsq")
            dsum = st_p.tile([D, nwin_d], f32, tag="dsum")
            dsq = st_p.tile([D, nwin_d], f32, tag="dsq")

            # ---- gg sweep: mean_gg per window into mean_g arena
            for w in range(nwin_g):
                B = goffs[w + 1] - goffs[w]
                gm = meta_p.tile([P, Bg], i32, tag="gmeta", name="gmeta")
                nc.sync.dma_start(gm[:, :B], T["gmeta"][:, goffs[w]:goffs[w + 1]])
                agg = sweep(xg_t[li], cfg["grows_g"], F, gm,
                            wgroups(gkws, w, 0, ngg),
                            lambda: ps_p.tile([F, W], f32, tag="agg", name="agg"),
                            "gg")
                nc.vector.tensor_copy(out=mean_g[:F, w * W:(w + 1) * W],
                                      in_=agg[:])
            # ---- dg sweep + combine into og arena
            for w in range(nwin_g):
                ws = slice(w * W, (w + 1) * W)
                B = goffs[w + 1] - goffs[w]
                gm = meta_p.tile([P, Bg], i32, tag="gmeta", name="gmeta")
                nc.sync.dma_start(gm[:, :B], T["gmeta"][:, goffs[w]:goffs[w + 1]])
                agg = sweep(xd_t[li], cfg["grows_d"], F, gm,
                            wgroups(gkws, w, ngg, len(gkws)),
                            lambda: ps_p.tile([F, W], f32, tag="agg", name="agg"),
                            "dg")
                mdg = sm_p.tile([F, W], f32, tag="mdg")
                nc.vector.tensor_copy(out=mdg[:], in_=agg[:])
                if li == 0:
                    xsl = sm_p.tile([F, W], f32, tag="xsl")
                    nc.sync.dma_start(xsl[:], T["xgt0"][:, ws])
                    xrhs = xsl[:]
                else:
                    xrhs = cur_g[:F, ws]
                out = ps_p.tile([D, W], f32, tag="out")
                nc.tensor.matmul(out[:], lhsT=wv64(f"wl_gg{li}", F),
                                 rhs=mean_g[:F, ws], start=True, stop=False)
                nc.tensor.matmul(out[:], lhsT=wv64(f"wl_dg{li}", F),
                                 rhs=mdg[:], start=False, stop=False)
                nc.tensor.matmul(out[:], lhsT=wv64(f"wr_g{li}", F),
                                 rhs=xrhs, start=False, stop=True)
                ncols = cfg["last_g"] if w == nwin_g - 1 else W
                stats_cols(og, ws, w, wv(f"bg{li}")[:D, :], out, gsum, gsq,
                           ncols)
            if li == 0 and "dbg_mean" in T:
                nc.sync.dma_start(out=T["dbg_mean"], in_=mean_g[:, :])
                nc.sync.dma_start(out=T["dbg_og"], in_=og[:D, :])
                nc.sync.dma_start(out=T["dbg_sum"], in_=gsum[:])
                nc.sync.dma_start(out=T["dbg_sq"], in_=gsq[:])
            # ---- og finalize (overlappable with gd sweep by scheduler)
            bn_normalize(og, AG, gsum, gsq, nwin_g, cfg["NG"],
                         wv(f"gam_g{li}"), wv(f"bet_g{li}"),
                         cur_g if li > 0 else None)
            if li < 2:
                gsh = dram.tile([AG, D], f32, tag="gshard")
                transpose_out(og, 0, AG, gsh[:])
                nc.gpsimd.collective_compute(
                    "AllGather", mybir.AluOpType.bypass, replica_groups=rg,
                    ins=[gsh[:].opt()], outs=[xg_t[li + 1].opt()])

            # ---- gd sweep (dis dst; reads layer-li gene table)
            for w in range(nwin_d):
                ws = slice(w * W, (w + 1) * W)
                B = doffs[w + 1] - doffs[w]
                dm = meta_p.tile([P, Bd], i32, tag="dmeta", name="dmeta")
                nc.sync.dma_start(dm[:, :B], T["dmeta"][:, doffs[w]:doffs[w + 1]])
                agg = sweep(xg_t[li], cfg["grows_g"], F, dm,
                            wgroups(dkws, w, 0, len(dkws)),
                            lambda: ps_p.tile([F, W], f32, tag="agg", name="agg"),
                            "gd")
                mgd = sm_p.tile([F, W], f32, tag="mdg")
                nc.vector.tensor_copy(out=mgd[:], in_=agg[:])
                if li == 0:
                    xsl = sm_p.tile([F, W], f32, tag="xsl")
                    nc.sync.dma_start(xsl[:], T["xdt0"][:, ws])
                    xrhs = xsl[:]
                else:
                    xrhs = cur_d[:F, ws]
                out = ps_p.tile([D, W], f32, tag="out")
                nc.tensor.matmul(out[:], lhsT=wv64(f"wl_gd{li}", F),
                                 rhs=mgd[:], start=True, stop=False)
                nc.tensor.matmul(out[:], lhsT=wv64(f"wr_d{li}", F),
                                 rhs=xrhs, start=False, stop=True)
                ncols = cfg["last_d"] if w == nwin_d - 1 else W
                stats_cols(od, ws, w, wv(f"bd{li}")[:D, :], out, dsum, dsq,
                           ncols)
            bn_normalize(od, AD, dsum, dsq, nwin_d, cfg["ND"],
                         wv(f"gam_d{li}"), wv(f"bet_d{li}"),
                         cur_d if li > 0 else None)
            if li < 2:
                dsh = dram.tile([AD, D], f32, tag="dshard")
                transpose_out(od, 0, AD, dsh[:])
                nc.gpsimd.collective_compute(
                    "AllGather", mybir.AluOpType.bypass, replica_groups=rg,
                    ins=[dsh[:].opt()], outs=[xd_t[li + 1].opt()])
            cur_g, cur_d = og, od

        # ------------------------------------------------------------------
        # post MLP: h = lrelu(bn(x@W1+b1)); out = h@W2+b2
        hg, hd = gB, dB
        gsum = st_p.tile([D, nwin_g], f32, tag="gsum")
        gsq = st_p.tile([D, nwin_g], f32, tag="gsq")
        dsum = st_p.tile([D, nwin_d], f32, tag="dsum")
        dsq = st_p.tile([D, nwin_d], f32, tag="dsq")
        for w in range(nwin_g):
            ws = slice(w * W, (w + 1) * W)
            out = ps_p.tile([D, W], f32, tag="out")
            nc.tensor.matmul(out[:], lhsT=wv64("w1g"), rhs=cur_g[:D, ws],
                             start=True, stop=True)
            ncols = cfg["last_g"] if w == nwin_g - 1 else W
            stats_cols(hg, ws, w, wv("b1g")[:D, :], out, gsum, gsq, ncols)
        for w in range(nwin_d):
            ws = slice(w * W, (w + 1) * W)
            out = ps_p.tile([D, W], f32, tag="out")
            nc.tensor.matmul(out[:], lhsT=wv64("w1d"), rhs=cur_d[:D, ws],
                             start=True, stop=True)
            ncols = cfg["last_d"] if w == nwin_d - 1 else W
            stats_cols(hd, ws, w, wv("b1d")[:D, :], out, dsum, dsq, ncols)
        bn_normalize(hg, AG, gsum, gsq, nwin_g, cfg["NG"],
                     wv("gam_pg"), wv("bet_pg"), None)
        bn_normalize(hd, AD, dsum, dsq, nwin_d, cfg["ND"],
                     wv("gam_pd"), wv("bet_pd"), None)
        # lin2 into mean_g arena (gene rows 0:64; dis rows 64:128)
        for w in range(nwin_g):
            ws = slice(w * W, (w + 1) * W)
            out = ps_p.tile([D, W], f32, tag="out")
            nc.tensor.matmul(out[:], lhsT=wv64("w2g"), rhs=hg[:D, ws],
                             start=True, stop=True)
            nc.vector.tensor_scalar(out=mean_g[:D, ws], in0=out[:],
                                    scalar1=wv("b2g")[:D, :], scalar2=None,
                                    op0=OP.add)
        for w in range(nwin_d):
            ws = slice(w * W, (w + 1) * W)
            out = ps_p.tile([D, W], f32, tag="out")
            nc.tensor.matmul(out[:], lhsT=wv64("w2d"), rhs=hd[:D, ws],
                             start=True, stop=True)
            nc.vector.tensor_scalar(out=dA[:D, ws], in0=out[:],
                                    scalar1=wv("b2d")[:D, :], scalar2=None,
                                    op0=OP.add)
        transpose_out(mean_g, 0, AG, T["og_out"])
        transpose_out(dA, 0, AD, T["od_out"])


# ----------------------------------------------------------------------------
# Entry point
# ----------------------------------------------------------------------------

def kernel(x_gene, x_dis, e_gg, e_gd, e_dg, params):
    import concourse.bacc as bacc
    import concourse.tile as tile
    import concourse.mybir as mybir
    import concourse.bass_utils as bass_utils

    import time
    _t0 = time.time()

    def _log(msg):
        print(f"[kernel +{time.time()-_t0:7.1f}s] {msg}", flush=True)

    x_gene = np.asarray(x_gene)
    x_dis = np.asarray(x_dis)
    cfg, host = _prep(x_gene, x_dis, e_gg, e_gd, e_dg, params)
    _log(f"prep done sum_k gene={sum(sum(k) for k in cfg['gkws'])} "
         f"dis={sum(sum(k) for k in cfg['dkws'])}")

    f32 = mybir.dt.float32
    i32 = mybir.dt.int32
    nc = bacc.Bacc("TRN2", target_bir_lowering=False, debug=False,
                   enable_asserts=False, num_devices=C)

    T = {}
    T["xg0"] = nc.dram_tensor("xg0", list(host["xg0"].shape), f32,
                              kind="ExternalInput").ap()
    T["xd0"] = nc.dram_tensor("xd0", list(host["xd0"].shape), f32,
                              kind="ExternalInput").ap()
    T["xgt0"] = nc.dram_tensor("xgt0", list(host["xgt0"].shape[1:]), f32,
                               kind="ExternalInput").ap()
    T["xdt0"] = nc.dram_tensor("xdt0", list(host["xdt0"].shape[1:]), f32,
                               kind="ExternalInput").ap()
    T["gmeta"] = nc.dram_tensor("gmeta", list(host["gmeta"].shape[1:]), i32,
                                kind="ExternalInput").ap()
    T["dmeta"] = nc.dram_tensor("dmeta", list(host["dmeta"].shape[1:]), i32,
                                kind="ExternalInput").ap()
    T["wall"] = nc.dram_tensor("wall", list(host["wall"].shape), f32,
                               kind="ExternalInput").ap()
    T["og_out"] = nc.dram_tensor("og_out", [cfg["nwin_g"] * W, cfg["D"]], f32,
                                 kind="ExternalOutput").ap()
    T["od_out"] = nc.dram_tensor("od_out", [cfg["nwin_d"] * W, cfg["D"]], f32,
                                 kind="ExternalOutput").ap()
    dbg = os.environ.get("BASSGNN_DBG") == "1"
    if dbg:
        AG = cfg["nwin_g"] * W
        T["dbg_mean"] = nc.dram_tensor("dbg_mean", [P, AG], f32,
                                       kind="ExternalOutput").ap()
        T["dbg_og"] = nc.dram_tensor("dbg_og", [cfg["D"], AG], f32,
                                     kind="ExternalOutput").ap()
        T["dbg_sum"] = nc.dram_tensor("dbg_sum", [cfg["D"], cfg["nwin_g"]], f32,
                                      kind="ExternalOutput").ap()
        T["dbg_sq"] = nc.dram_tensor("dbg_sq", [cfg["D"], cfg["nwin_g"]], f32,
                                     kind="ExternalOutput").ap()
        T["dbg_rstat"] = nc.dram_tensor("dbg_rstat", [cfg["D"], 16], f32,
                                        kind="ExternalOutput").ap()
        T["dbg_var"] = nc.dram_tensor("dbg_var", [cfg["D"], 8], f32,
                                      kind="ExternalOutput").ap()

    with tile.TileContext(nc) as tc:
        _build(nc, tc, cfg, T)
    _log(f"trace+schedule done: {sum(len(b.instructions) for b in nc.main_func.blocks)} instructions")
    nc.compile()
    _log("bacc compile done")

    in_maps = []
    for c in range(C):
        in_maps.append({
            "xg0": host["xg0"], "xd0": host["xd0"],
            "xgt0": host["xgt0"][c], "xdt0": host["xdt0"][c],
            "gmeta": host["gmeta"][c], "dmeta": host["dmeta"][c],
            "wall": host["wall"],
        })

    if os.environ.get("BASSGNN_SIM") == "1":
        from concourse.bass_interp import MultiCoreSim
        sim = MultiCoreSim(nc, num_cores=C, trace=False,
                           require_finite=False, require_nnan=False)
        cores = list(sim.cores.values())
        for c, core in enumerate(cores):
            for k, v in in_maps[c].items():
                core.tensor(k)[:] = v
        try:
            sim.simulate(check_with_hw=False)
        except Exception as e:
            print("SIM EXCEPTION:", type(e).__name__, str(e)[:300])
        keys = ["og_out", "od_out"]
        if dbg:
            keys += ["dbg_mean", "dbg_og", "dbg_sum", "dbg_sq",
                     "dbg_rstat", "dbg_var"]
        results = [{k: np.array(core.tensor(k)) for k in keys}
                   for core in cores]
        kernel.debug_results = results
    else:
        _t1 = time.time()
        try:
            r = bass_utils.run_bass_kernel_spmd(
                nc, in_maps, core_ids=list(range(C)),
                trace=os.environ.get("BASSGNN_TRACE") == "1")
        except ModuleNotFoundError:
            r = bass_utils.run_bass_kernel_spmd(
                nc, in_maps, core_ids=list(range(C)), trace=False)
        _log(f"spmd run done (wall {time.time()-_t1:.1f}s, "
             f"exec_time_ns={r.exec_time_ns})")
        kernel.last_exec_time_ns = r.exec_time_ns
        results = r.results
        nrep = int(os.environ.get("BASSGNN_BENCH", "0"))
        if nrep:
            kernel.bench_ns = _bench(nc, in_maps, nrep)

    all_g = np.concatenate([results[c]["og_out"] for c in range(C)], axis=0)
    all_d = np.concatenate([results[c]["od_out"] for c in range(C)], axis=0)
    out_g = all_g[host["gpack"]].astype(np.float32)
    out_d = all_d[host["dpack"]].astype(np.float32)
    return out_g, out_d


def _bench(nc, in_maps, nrep):
    """Build the PJRT callable once, run nrep times, return min wall ns."""
    import time
    import jax
    import numpy as np
    from jax.sharding import Mesh, PartitionSpec
    from jax.experimental.shard_map import shard_map
    import concourse.bass2jax as b2j
    import concourse.mybir as mybir

    b2j.install_neuronx_cc_hook()
    partition_name = (nc.partition_id_tensor.name
                      if nc.partition_id_tensor else None)
    in_names, out_names, out_avals, zero_outs = [], [], [], []
    for alloc in nc.m.functions[0].allocations:
        if not isinstance(alloc, mybir.MemoryLocationSet):
            continue
        name = alloc.memorylocations[0].name
        if alloc.kind == "ExternalInput":
            if name != partition_name:
                in_names.append(name)
        elif alloc.kind == "ExternalOutput":
            out_names.append(name)
            shape = tuple(alloc.tensor_shape)
            dtype = mybir.dt.np(alloc.dtype)
            out_avals.append(jax.core.ShapedArray(shape, dtype))
            zero_outs.append(np.zeros(shape, dtype))
    n_params = len(in_names)
    n_outs = len(out_avals)
    in_names = in_names + out_names
    if partition_name is not None:
        in_names.append(partition_name)
    donate = tuple(range(n_params, n_params + n_outs))

    def _body(*args):
        operands = list(args)
        if partition_name is not None:
            operands.append(b2j.partition_id_tensor())
        outs = b2j._bass_exec_p.bind(
            *operands, out_avals=tuple(out_avals), in_names=tuple(in_names),
            out_names=tuple(out_names), lowering_input_output_aliases=(),
            sim_require_finite=True, sim_require_nnan=True, nc=nc)
        return tuple(outs)

    devices = jax.devices()[:C]
    mesh = Mesh(np.asarray(devices), ("core",))
    sharded = jax.jit(
        shard_map(_body, mesh=mesh,
                  in_specs=(PartitionSpec("core"),) * (n_params + n_outs),
                  out_specs=(PartitionSpec("core"),) * n_outs,
                  check_rep=False),
        keep_unused=True)
    concat_in = [
        np.concatenate([np.asarray(in_maps[c][n]) for c in range(C)], axis=0)
        for n in in_names[:n_params]]
    concat_zeros = [np.zeros((C * z.shape[0], *z.shape[1:]), z.dtype)
                    for z in zero_outs]
    args_dev = jax.device_put(concat_in + concat_zeros)
    out = sharded(*args_dev)
    jax.block_until_ready(out)
    times = []
    for _ in range(nrep):
        t0 = time.perf_counter()
        out = sharded(*args_dev)
        jax.block_until_ready(out)
        times.append(time.perf_counter() - t0)
    times = np.array(times) * 1e9
    print(f"[bench] n={nrep} min={times.min()/1e6:.3f}ms "
          f"p50={np.percentile(times,50)/1e6:.3f}ms "
          f"mean={times.mean()/1e6:.3f}ms", flush=True)
    return float(times.min())


kernel.last_exec_time_ns = None


# revision 25
# speedup vs baseline: 1.1334x; 1.1334x over previous
"""Trainium2 Bass kernel for a 3-layer hetero GraphSAGE encoder (gene/disease).

Strategy (8 NeuronCores, SPMD):
  - dst-nodes are bin-packed into (core, window-of-64) bins, balanced by degree;
    every edge lives with its dst node, so segment-mean is core-local.
  - Per window: dma_gather of src feature rows (fp32) from a DRAM table
    (int16 indices per <=32768-row table group, single_packet=False),
    one-hot matrices built on DVE, and a TensorE matmul G.T @ S that
    accumulates the segment MEAN (1/cnt folded into the one-hot) in PSUM,
    feature-major [d, 64].
  - Small matmuls apply Wl/Wr, bias via DVE (stats fused via accum_out).
  - BN stats AllReduce'd across the 8 cores; normalize + LeakyReLU + residual
    on a feature-major SBUF arena; PE transpose back to node-major; AllGather
    rebuilds the next layer's full gather table on every core.
"""

import os
import numpy as np

C = 8          # cores
W = 64         # dst window (nodes per bin)
P = 128        # partitions / edges per matmul sub-block
SLOPE = 0.01
EPS = 1e-5
PIECE_BYTES = 7 * 1024   # max gather-piece bytes per partition


# ----------------------------------------------------------------------------
# Host-side preprocessing
# ----------------------------------------------------------------------------

def _snake_pack(weights, n_bins, caps):
    """Assign items to bins (cap[b] items each) balancing sum(weights).

    Deals items in descending weight order, snaking across bins that still
    have capacity. Returns assign[item] = bin id.
    """
    n = len(weights)
    assert caps.sum() == n
    order = np.argsort(-weights, kind="stable")
    assign = np.empty(n, np.int64)
    used = np.zeros(n_bins, np.int64)
    pos = 0
    rnd = 0
    while pos < n:
        active = np.nonzero(used < caps)[0]
        if rnd % 2:
            active = active[::-1]
        take = min(len(active), n - pos)
        sel = active[:take]
        assign[order[pos:pos + take]] = sel
        used[sel] += 1
        pos += take
        rnd += 1
    return assign


MAX_GROUP_ROWS = 32768   # int16 index limit for dma_gather


def _pack_edge_meta(src_packed, dst_bin, dst_slot, scale, n_bins, n_src):
    """Organize edges per (bin, src-group); return per-group sorted arrays.

    Edges are grouped by src table group (each <= MAX_GROUP_ROWS rows so an
    int16 relative index addresses it). Returns (per_group, counts, grows):
    per_group[h] = (bin_id, rel_src i16, dst_slot, scale) sorted by bin;
    counts [n_bins, H].
    """
    H = max(1, -(-n_src // MAX_GROUP_ROWS))
    grows = -(-n_src // H)
    grp = src_packed // grows
    key = dst_bin * H + grp
    order = np.argsort(key, kind="stable")
    kk, ss, ssl, ssc = (key[order], src_packed[order], dst_slot[order],
                        scale[order])
    counts = np.bincount(kk, minlength=n_bins * H).reshape(n_bins, H)
    per_group = []
    for h in range(H):
        sel = (kk % H) == h
        per_group.append((kk[sel] // H, (ss[sel] - h * grows).astype(np.int16),
                          ssl[sel].astype(np.float32),
                          ssc[sel].astype(np.float32)))
    return per_group, counts, grows


def _window_meta(metas, nwin):
    """Build per-core meta [C, 128, total_cols] with per-window k's.

    metas: list over groups g of (per_group-tuple, counts [nbins, .]) entries
    flattened: each entry = (bin_id, rel, slot, sc, counts_col).
    Returns (arr, kws, offs): kws[g][w] sub-blocks for group g in window w
    (max over cores); offs[w] = column offset of window w's block.
    """
    nbins = nwin * C
    kws = []
    for (b, rel, slot, sc, cnt, gbase) in metas:
        grid = cnt.reshape(C, nwin)
        kws.append(np.maximum(1, -(-grid.max(axis=0) // P)))
    offs = np.zeros(nwin + 1, np.int64)
    for w in range(nwin):
        offs[w + 1] = offs[w] + 7 * sum(int(k[w]) for k in kws)
    arr = np.zeros((C, P, int(offs[-1])), np.int32)
    for g, (b, rel, slot, sc, cnt, gbase) in enumerate(metas):
        # position of each edge within its (bin, group)
        o = np.zeros(nbins + 1, np.int64)
        np.cumsum(cnt.reshape(-1), out=o[1:])
        pos = np.arange(len(b)) - o[b]
        core = b // nwin
        w = b % nwin
        for ww in range(nwin):
            base = offs[ww] + 7 * sum(int(k[ww]) for k in kws[:g])
            sel = w == ww
            if not sel.any():
                continue
            kk = int(kws[g][ww])
            i = pos[sel]
            cc = core[sel]
            idx16 = np.zeros((C, 16, kk * 8), np.int16)
            idx16[cc, i % 16, i // 16] = rel[sel]
            idx16 = np.tile(idx16, (1, 8, 1)).view(np.int32)
            arr[:, :, base:base + 4 * kk] = idx16
            # absolute int32 indices, p-major (for indirect_dma_start)
            iv = np.zeros((C, P, kk), np.int32)
            iv[cc, i % P, i // P] = (rel[sel].astype(np.int32) + gbase)
            arr[:, :, base + 4 * kk:base + 5 * kk] = iv
            dv = np.zeros((C, P, kk), np.float32)
            scv = np.zeros((C, P, kk), np.float32)
            dv[cc, i % P, i // P] = slot[sel]
            scv[cc, i % P, i // P] = sc[sel]
            arr[:, :, base + 5 * kk:base + 6 * kk] = dv.view(np.int32)
            arr[:, :, base + 6 * kk:base + 7 * kk] = scv.view(np.int32)
    return arr, kws, offs


def _prep(x_gene, x_dis, e_gg, e_gd, e_dg, params):
    NG, DIN = x_gene.shape
    ND = x_dis.shape[0]
    D = 64
    assert NG % C == 0 and ND % C == 0
    npc_g, npc_d = NG // C, ND // C                  # real nodes per core
    nwin_g, nwin_d = -(-npc_g // W), -(-npc_d // W)  # windows per core
    last_g = npc_g - (nwin_g - 1) * W                # real cols in last window
    last_d = npc_d - (nwin_d - 1) * W
    NGP, NDP = C * nwin_g * W, C * nwin_d * W        # padded table sizes

    e_gg = np.asarray(e_gg, np.int64)
    e_gd = np.asarray(e_gd, np.int64)
    e_dg = np.asarray(e_dg, np.int64)

    deg_gg = np.bincount(e_gg[1], minlength=NG).astype(np.float64)
    deg_dg = np.bincount(e_dg[1], minlength=NG).astype(np.float64)
    deg_gd = np.bincount(e_gd[1], minlength=ND).astype(np.float64)

    # ---- bin-pack gene nodes into C*nwin_g bins (cap W, last window smaller)
    nbins_g = C * nwin_g
    caps_g = np.full(nbins_g, W, np.int64)
    caps_g[np.arange(C) * nwin_g + (nwin_g - 1)] = last_g
    wsum_g = deg_gg / max(deg_gg.mean(), 1e-9) + deg_dg / max(deg_dg.mean(), 1e-9)
    bin_g = _snake_pack(wsum_g, nbins_g, caps_g)

    nbins_d = C * nwin_d
    caps_d = np.full(nbins_d, W, np.int64)
    caps_d[np.arange(C) * nwin_d + (nwin_d - 1)] = last_d
    bin_d = _snake_pack(deg_gd.astype(np.float64), nbins_d, caps_d)

    # slot within bin, packed id
    def slots(bin_assign, n_bins):
        order = np.argsort(bin_assign, kind="stable")
        counts = np.bincount(bin_assign, minlength=n_bins)
        offs = np.zeros(n_bins + 1, np.int64)
        np.cumsum(counts, out=offs[1:])
        slot = np.empty_like(bin_assign)
        slot[order] = np.arange(len(bin_assign)) - offs[bin_assign[order]]
        return slot

    slot_g = slots(bin_g, nbins_g)
    slot_d = slots(bin_d, nbins_d)
    gpack = bin_g * W + slot_g      # packed id in [0, NGP)
    dpack = bin_d * W + slot_d

    # ---- edge meta per type --------------------------------------------------
    scale_gg = (1.0 / np.maximum(deg_gg, 1.0))[e_gg[1]]
    scale_dg = (1.0 / np.maximum(deg_dg, 1.0))[e_dg[1]]
    scale_gd = (1.0 / np.maximum(deg_gd, 1.0))[e_gd[1]]

    def meta(e, scale, src_pack, dst_bin_assign, dst_slot, n_bins, n_src):
        sbin = dst_bin_assign[e[1]]
        sslot = dst_slot[e[1]]
        return _pack_edge_meta(src_pack[e[0]], sbin, sslot, scale, n_bins,
                               n_src)

    m_gg, c_gg, grows_g = meta(e_gg, scale_gg, gpack, bin_g, slot_g, nbins_g,
                               NGP)
    m_dg, c_dg, grows_d = meta(e_dg, scale_dg, dpack, bin_g, slot_g, nbins_g,
                               NDP)
    m_gd, c_gd, _ = meta(e_gd, scale_gd, gpack, bin_d, slot_d, nbins_d, NGP)

    gentries = ([m + (c_gg[:, h], h * grows_g) for h, m in enumerate(m_gg)]
                + [m + (c_dg[:, h], h * grows_d) for h, m in enumerate(m_dg)])
    dentries = [m + (c_gd[:, h], h * grows_g) for h, m in enumerate(m_gd)]
    gmeta, gkws, goffs = _window_meta(gentries, nwin_g)
    dmeta, dkws, doffs = _window_meta(dentries, nwin_d)
    n_gg_groups = len(m_gg)

    # ---- packed feature tables / transposed shards --------------------------
    xg0 = np.zeros((NGP, DIN), np.float32)
    xg0[gpack] = np.asarray(x_gene, np.float32)
    xd0 = np.zeros((NDP, DIN), np.float32)
    xd0[dpack] = np.asarray(x_dis, np.float32)
    xgt0 = np.ascontiguousarray(
        xg0.reshape(C, nwin_g * W, DIN).transpose(0, 2, 1))  # [C, DIN, nwin*W]
    xdt0 = np.ascontiguousarray(
        xd0.reshape(C, nwin_d * W, DIN).transpose(0, 2, 1))

    # ---- weights wall [128, NW] ---------------------------------------------
    cols = []

    def f32(a):
        return np.asarray(a, np.float32)

    def padP(a):   # pad [F, c] to [128, c]
        a = f32(a)
        out = np.zeros((P, a.shape[1]), np.float32)
        out[: a.shape[0]] = a
        return out

    layout = {}

    def add(name, arr):
        layout[name] = (sum(c.shape[1] for c in cols), arr.shape[1])
        cols.append(padP(arr))

    add("I", np.eye(P, dtype=np.float32))
    add("J", np.tile(np.arange(W, dtype=np.float32)[None, :], (P, 1)))
    L = params["layers"]
    for li in range(3):
        p = L[li]
        add(f"wl_gg{li}", 0.5 * f32(p["gg"]["Wl"]))
        add(f"wl_dg{li}", 0.5 * f32(p["dg"]["Wl"]))
        add(f"wl_gd{li}", f32(p["gd"]["Wl"]))
        add(f"wr_g{li}", 0.5 * (f32(p["gg"]["Wr"]) + f32(p["dg"]["Wr"])))
        add(f"wr_d{li}", f32(p["gd"]["Wr"]))
        add(f"bg{li}", (0.5 * (f32(p["gg"]["b"]) + f32(p["dg"]["b"])))[:, None])
        add(f"bd{li}", f32(p["gd"]["b"])[:, None])
        add(f"gam_g{li}", f32(p["bn_gene"]["g"])[:, None])
        add(f"bet_g{li}", f32(p["bn_gene"]["b"])[:, None])
        add(f"gam_d{li}", f32(p["bn_dis"]["g"])[:, None])
        add(f"bet_d{li}", f32(p["bn_dis"]["b"])[:, None])
    for nt in ("gene", "dis"):
        q = params["post"][nt]
        s = "g" if nt == "gene" else "d"
        add(f"w1{s}", f32(q["lin1"]["W"]))
        add(f"w2{s}", f32(q["lin2"]["W"]))
        add(f"b1{s}", f32(q["lin1"]["b"])[:, None])
        add(f"b2{s}", f32(q["lin2"]["b"])[:, None])
        add(f"gam_p{s}", f32(q["bn"]["g"])[:, None])
        add(f"bet_p{s}", f32(q["bn"]["b"])[:, None])
    wall = np.concatenate(cols, axis=1)

    cfg = dict(
        NG=NG, ND=ND, DIN=DIN, D=D, NGP=NGP, NDP=NDP,
        npc_g=npc_g, npc_d=npc_d, nwin_g=nwin_g, nwin_d=nwin_d,
        last_g=last_g, last_d=last_d,
        gkws=[k.tolist() for k in gkws], dkws=[k.tolist() for k in dkws],
        goffs=goffs.tolist(), doffs=doffs.tolist(),
        n_gg_groups=n_gg_groups,
        grows_g=grows_g, grows_d=grows_d,
        wall_cols=wall.shape[1], layout=layout,
    )
    host = dict(xg0=xg0, xd0=xd0, xgt0=xgt0, xdt0=xdt0,
                gmeta=gmeta, dmeta=dmeta, wall=wall,
                gpack=gpack, dpack=dpack)
    return cfg, host


# ----------------------------------------------------------------------------
# Device program
# ----------------------------------------------------------------------------

def _pieces(k, F):
    npc = max(1, -(-(k * F * 4) // PIECE_BYTES))
    npc = min(npc, k)
    bounds = np.linspace(0, k, npc + 1).astype(int)
    return [(int(a), int(b)) for a, b in zip(bounds[:-1], bounds[1:]) if b > a]


def _build(nc, tc, cfg, T):
    import concourse.bass as bass
    import concourse.mybir as mybir
    from contextlib import ExitStack

    f32 = mybir.dt.float32
    i32 = mybir.dt.int32
    i16 = mybir.dt.int16
    OP = mybir.AluOpType
    AF = mybir.ActivationFunctionType

    D = cfg["D"]
    DIN = cfg["DIN"]
    nwin_g, nwin_d = cfg["nwin_g"], cfg["nwin_d"]
    gkws, dkws = cfg["gkws"], cfg["dkws"]
    goffs, doffs = cfg["goffs"], cfg["doffs"]
    ngg = cfg["n_gg_groups"]
    Bg = max(goffs[w + 1] - goffs[w] for w in range(nwin_g))
    Bd = max(doffs[w + 1] - doffs[w] for w in range(nwin_d))
    AG = nwin_g * W     # arena cols gene
    AD = nwin_d * W
    lay = cfg["layout"]

    ctx = ExitStack()
    with ctx:
        consts = ctx.enter_context(tc.tile_pool(name="consts", bufs=1))
        arenas = ctx.enter_context(tc.tile_pool(name="arenas", bufs=1))
        meta_p = ctx.enter_context(tc.tile_pool(name="meta", bufs=3))
        gat_p = ctx.enter_context(tc.tile_pool(
            name="gather", bufs=int(os.environ.get("BASSGNN_GBUFS", "2"))))
        s_p = ctx.enter_context(tc.tile_pool(
            name="onehot", bufs=int(os.environ.get("BASSGNN_SBUFS", "2"))))
        sm_p = ctx.enter_context(tc.tile_pool(name="small", bufs=2))
        st_p = ctx.enter_context(tc.tile_pool(name="stats", bufs=1))
        ps_p = ctx.enter_context(tc.tile_pool(name="psum", bufs=2, space="PSUM"))
        dram = ctx.enter_context(tc.tile_pool(name="dram", bufs=1, space="DRAM"))

        wall = consts.tile([P, cfg["wall_cols"]], f32)
        nc.sync.dma_start(wall[:], T["wall"][:, :])

        def wv(name):
            o, n = lay[name]
            return wall[:, o:o + n]

        def wv64(name, F=D):
            o, n = lay[name]
            return wall[:F, o:o + n]

        I64 = wall[:D, lay["I"][0]:lay["I"][0] + D]
        J = wv("J")  # [128, W] f32 iota row

        # persistent arenas (feature-major)
        gA = arenas.tile([P, AG], f32, tag="gA")
        gB = arenas.tile([P, AG], f32, tag="gB")
        dA = arenas.tile([P, AD], f32, tag="dA")
        dB = arenas.tile([P, AD], f32, tag="dB")
        mean_g = arenas.tile([P, AG], f32, tag="mean")

        # next-layer gather tables (AllGather outputs, Shared)
        xg_t = [T["xg0"]]
        xd_t = [T["xd0"]]
        for li in (1, 2):
            xg_t.append(dram.tile([cfg["NGP"], D], f32, tag=f"xg{li}",
                                  name=f"xgtab{li}", addr_space="Shared")[:])
            xd_t.append(dram.tile([cfg["NDP"], D], f32, tag=f"xd{li}",
                                  name=f"xdtab{li}", addr_space="Shared")[:])

        rg = [list(range(C))]

        def sweep(table, grows, F, meta_tile, groups, dst_psum_fn, typ):
            """Aggregation for one window: per src-group gather+one-hot+matmul.

            groups: list of (k_subblocks, meta col offset, src group h).
            """
            agg = dst_psum_fn()
            ktot = sum(k for k, _, _ in groups)
            jglob = 0
            for (k, off, h) in groups:
                idx16_ap = meta_tile[:, off:off + 4 * k].bitcast(i16)
                idx32_ap = meta_tile[:, off + 4 * k:off + 5 * k]
                dstv_ap = meta_tile[:, off + 5 * k:off + 6 * k].bitcast(f32)
                sc_ap = meta_tile[:, off + 6 * k:off + 7 * k].bitcast(f32)
                rows = min(grows, table.shape[0] - h * grows)
                G = gat_p.tile([P, k * F], f32, tag=f"g_{typ}",
                               name=f"g_{typ}")
                if F == 64 and os.environ.get("BASSGNN_INDIRECT", "0") == "1":
                    # one-idx-per-partition indirect gather: 25ns/row for
                    # 256B rows (vs 38 for dma_gather)
                    for j in range(k):
                        nc.gpsimd.indirect_dma_start(
                            out=G[:, j * F:(j + 1) * F],
                            out_offset=None,
                            in_=table,
                            in_offset=bass.IndirectOffsetOnAxis(
                                ap=idx32_ap[:, j:j + 1], axis=0),
                        )
                else:
                    nc.gpsimd.dma_gather(
                        out_ap=G[:].rearrange("p (k f) -> p k f", f=F),
                        in_ap=table[h * grows:h * grows + rows, :],
                        idxs_ap=idx16_ap,
                        num_idxs=k * P,
                        num_idxs_reg=k * P,
                        elem_size=F,
                        single_packet=False,
                    )
                S = s_p.tile([P, k * W], f32, tag=f"s_{typ}", name=f"s_{typ}")
                S3 = S[:].rearrange("p (k w) -> p k w", w=W)
                dv3 = dstv_ap.unsqueeze(2).broadcast_to([P, k, W])
                J3 = J.unsqueeze(1).broadcast_to([P, k, W])
                nc.vector.tensor_tensor(out=S3, in0=dv3, in1=J3, op=OP.is_equal)
                sc3 = sc_ap.unsqueeze(2).broadcast_to([P, k, W])
                nc.vector.tensor_tensor(out=S3, in0=S3, in1=sc3, op=OP.mult)
                for j in range(k):
                    nc.tensor.matmul(
                        agg[:],
                        lhsT=G[:, j * F:(j + 1) * F],
                        rhs=S[:, j * W:(j + 1) * W],
                        start=(jglob == 0), stop=(jglob == ktot - 1),
                    )
                    jglob += 1
            return agg

        def wgroups(kws, w, g0, g1):
            out = []
            off = 7 * sum(int(kws[g][w]) for g in range(g0))
            for g in range(g0, g1):
                k = int(kws[g][w])
                out.append((k, off, g - g0))
                off += 7 * k
            return out

        def stats_cols(arena, ws, w, bias_col, psum, ssum, ssq, ncols):
            """psum [D, W] + bias -> arena[:, ws]; accumulate sum/sumsq."""
            if ncols == W:
                nc.vector.tensor_scalar(
                    out=arena[:D, ws], in0=psum[:], scalar1=bias_col, scalar2=None,
                    op0=OP.add, op1=OP.add, accum_out=ssum[:, w:w + 1])
                sq = sm_p.tile([D, W], f32, tag="sq")
                nc.vector.scalar_tensor_tensor(
                    out=sq[:], in0=arena[:D, ws], scalar=1.0, in1=arena[:D, ws],
                    op0=OP.mult, op1=OP.mult, accum_out=ssq[:, w:w + 1])
            else:
                nc.vector.tensor_scalar(
                    out=arena[:D, ws], in0=psum[:], scalar1=bias_col, scalar2=None,
                    op0=OP.add)
                sl = arena[:D, ws.start:ws.start + ncols]
                part = sm_p.tile([D, W], f32, tag="sq")
                nc.vector.tensor_scalar(
                    out=part[:, :ncols], in0=sl, scalar1=0.0, scalar2=None,
                    op0=OP.add, op1=OP.add, accum_out=ssum[:, w:w + 1])
                nc.vector.scalar_tensor_tensor(
                    out=part[:, :ncols], in0=sl, scalar=1.0, in1=sl,
                    op0=OP.mult, op1=OP.mult, accum_out=ssq[:, w:w + 1])

        def bn_normalize(arena, ncols, ssum, ssq, nwin, n_real, gam, bet,
                         resid_arena, out_rows=D):
            """AllReduce stats; arena = lrelu(bn(arena)) (+ resid)."""
            st2 = sm_p.tile([D, 2], f32, tag="st2")
            nc.vector.tensor_reduce(out=st2[:, 0:1], in_=ssum[:],
                                    op=OP.add, axis=mybir.AxisListType.X)
            nc.vector.tensor_reduce(out=st2[:, 1:2], in_=ssq[:],
                                    op=OP.add, axis=mybir.AxisListType.X)
            cin = dram.tile([D, 2], f32, tag="cc_in")
            cout = dram.tile([D, 2], f32, tag="cc_out", addr_space="Shared")
            nc.sync.dma_start(cin[:], st2[:])
            nc.gpsimd.collective_compute(
                "AllReduce", OP.add, replica_groups=rg,
                ins=[cin[:].opt()], outs=[cout[:].opt()])
            rstat = sm_p.tile([D, 2], f32, tag="rstat")
            nc.sync.dma_start(rstat[:], cout[:])
            mcol = sm_p.tile([D, 1], f32, tag="mcol")
            vcol = sm_p.tile([D, 1], f32, tag="vcol")
            acol = sm_p.tile([D, 1], f32, tag="acol")
            bcol = sm_p.tile([D, 1], f32, tag="bcol")
            inv_n = 1.0 / float(n_real)
            nc.vector.tensor_scalar(out=mcol[:], in0=rstat[:, 0:1],
                                    scalar1=inv_n, scalar2=None, op0=OP.mult)
            # v = E[x^2] - m^2 + eps
            nc.vector.tensor_scalar(out=vcol[:], in0=rstat[:, 1:2],
                                    scalar1=inv_n, scalar2=None, op0=OP.mult)
            m2 = sm_p.tile([D, 1], f32, tag="m2")
            nc.vector.tensor_tensor(out=m2[:], in0=mcol[:], in1=mcol[:],
                                    op=OP.mult)
            nc.vector.tensor_tensor(out=vcol[:], in0=vcol[:], in1=m2[:],
                                    op=OP.subtract)
            nc.vector.tensor_scalar(out=vcol[:], in0=vcol[:], scalar1=EPS,
                                    scalar2=None, op0=OP.add)
            if "dbg_rstat" in T:
                i = T["_dbg_i"] = T.get("_dbg_i", -1) + 1
                nc.sync.dma_start(out=T["dbg_rstat"][:, 2 * i:2 * i + 2],
                                  in_=rstat[:])
                nc.sync.dma_start(out=T["dbg_var"][:, i:i + 1], in_=vcol[:])
            if os.environ.get("BASSGNN_NOSQRT") == "1":
                nc.vector.tensor_scalar(out=vcol[:], in0=vcol[:], scalar1=1.0,
                                        scalar2=None, op0=OP.max)
            else:
                nc.scalar.sqrt(out=vcol[:], in_=vcol[:])
            nc.vector.reciprocal(out=acol[:], in_=vcol[:])
            nc.vector.tensor_tensor(out=acol[:], in0=acol[:], in1=gam[:D, :],
                                    op=OP.mult)
            nc.vector.tensor_tensor(out=bcol[:], in0=mcol[:], in1=acol[:],
                                    op=OP.mult)
            nc.vector.tensor_tensor(out=bcol[:], in0=bet[:D, :], in1=bcol[:],
                                    op=OP.subtract)
            sl = arena[:out_rows, :ncols]
            nc.vector.tensor_scalar(out=sl, in0=sl, scalar1=acol[:],
                                    scalar2=bcol[:], op0=OP.mult, op1=OP.add)
            nc.vector.scalar_tensor_tensor(out=sl, in0=sl, scalar=SLOPE,
                                           in1=sl, op0=OP.mult, op1=OP.max)
            if resid_arena is not None:
                nc.vector.tensor_tensor(out=sl, in0=sl,
                                        in1=resid_arena[:out_rows, :ncols],
                                        op=OP.add)

        def transpose_out(arena, rows0, ncols, dst_dram):
            """arena[rows0:rows0+64, :ncols] -> dst_dram [ncols, 64] node-major."""
            nch = -(-ncols // P)
            for c in range(nch):
                cs = min(P, ncols - c * P)
                tp = ps_p.tile([P, D], f32, tag="tp")
                nc.tensor.transpose(
                    out=tp[:cs, :],
                    in_=arena[rows0:rows0 + D, c * P:c * P + cs],
                    identity=I64)
                stg = sm_p.tile([P, D], f32, tag="tstage")
                nc.vector.tensor_copy(out=stg[:cs, :], in_=tp[:cs, :])
                nc.sync.dma_start(out=dst_dram[c * P:c * P + cs, :],
                                  in_=stg[:cs, :])

        # ------------------------------------------------------------------
        cur_g, cur_d = None, None          # feature-major arenas of layer input
        for li in range(3):
            F = DIN if li == 0 else D
            og = [gA, gB, gA][li]
            od = [dA, dB, dA][li]
            gsum = st_p.tile([D, nwin_g], f32, tag="gsum")
            gsq = st_p.tile([D, nwin_g], f32, tag="gsq")
            dsum = st_p.tile([D, nwin_d], f32, tag="dsum")
            dsq = st_p.tile([D, nwin_d], f32, tag="dsq")

            # ---- gg sweep: mean_gg per window into mean_g arena
            for w in range(nwin_g):
                B = goffs[w + 1] - goffs[w]
                gm = meta_p.tile([P, Bg], i32, tag="gmeta", name="gmeta")
                nc.sync.dma_start(gm[:, :B], T["gmeta"][:, goffs[w]:goffs[w + 1]])
                agg = sweep(xg_t[li], cfg["grows_g"], F, gm,
                            wgroups(gkws, w, 0, ngg),
                            lambda: ps_p.tile([F, W], f32, tag="agg", name="agg"),
                            "gg")
                nc.vector.tensor_copy(out=mean_g[:F, w * W:(w + 1) * W],
                                      in_=agg[:])
            # ---- dg sweep + combine into og arena
            for w in range(nwin_g):
                ws = slice(w * W, (w + 1) * W)
                B = goffs[w + 1] - goffs[w]
                gm = meta_p.tile([P, Bg], i32, tag="gmeta", name="gmeta")
                nc.sync.dma_start(gm[:, :B], T["gmeta"][:, goffs[w]:goffs[w + 1]])
                agg = sweep(xd_t[li], cfg["grows_d"], F, gm,
                            wgroups(gkws, w, ngg, len(gkws)),
                            lambda: ps_p.tile([F, W], f32, tag="agg", name="agg"),
                            "dg")
                mdg = sm_p.tile([F, W], f32, tag="mdg")
                nc.vector.tensor_copy(out=mdg[:], in_=agg[:])
                if li == 0:
                    xsl = sm_p.tile([F, W], f32, tag="xsl")
                    nc.sync.dma_start(xsl[:], T["xgt0"][:, ws])
                    xrhs = xsl[:]
                else:
                    xrhs = cur_g[:F, ws]
                out = ps_p.tile([D, W], f32, tag="out")
                nc.tensor.matmul(out[:], lhsT=wv64(f"wl_gg{li}", F),
                                 rhs=mean_g[:F, ws], start=True, stop=False)
                nc.tensor.matmul(out[:], lhsT=wv64(f"wl_dg{li}", F),
                                 rhs=mdg[:], start=False, stop=False)
                nc.tensor.matmul(out[:], lhsT=wv64(f"wr_g{li}", F),
                                 rhs=xrhs, start=False, stop=True)
                ncols = cfg["last_g"] if w == nwin_g - 1 else W
                stats_cols(og, ws, w, wv(f"bg{li}")[:D, :], out, gsum, gsq,
                           ncols)
            if li == 0 and "dbg_mean" in T:
                nc.sync.dma_start(out=T["dbg_mean"], in_=mean_g[:, :])
                nc.sync.dma_start(out=T["dbg_og"], in_=og[:D, :])
                nc.sync.dma_start(out=T["dbg_sum"], in_=gsum[:])
                nc.sync.dma_start(out=T["dbg_sq"], in_=gsq[:])
            # ---- og finalize (overlappable with gd sweep by scheduler)
            bn_normalize(og, AG, gsum, gsq, nwin_g, cfg["NG"],
                         wv(f"gam_g{li}"), wv(f"bet_g{li}"),
                         cur_g if li > 0 else None)
            if li < 2:
                gsh = dram.tile([AG, D], f32, tag="gshard")
                transpose_out(og, 0, AG, gsh[:])
                nc.gpsimd.collective_compute(
                    "AllGather", mybir.AluOpType.bypass, replica_groups=rg,
                    ins=[gsh[:].opt()], outs=[xg_t[li + 1].opt()])

            # ---- gd sweep (dis dst; reads layer-li gene table)
            for w in range(nwin_d):
                ws = slice(w * W, (w + 1) * W)
                B = doffs[w + 1] - doffs[w]
                dm = meta_p.tile([P, Bd], i32, tag="dmeta", name="dmeta")
                nc.sync.dma_start(dm[:, :B], T["dmeta"][:, doffs[w]:doffs[w + 1]])
                agg = sweep(xg_t[li], cfg["grows_g"], F, dm,
                            wgroups(dkws, w, 0, len(dkws)),
                            lambda: ps_p.tile([F, W], f32, tag="agg", name="agg"),
                            "gd")
                mgd = sm_p.tile([F, W], f32, tag="mdg")
                nc.vector.tensor_copy(out=mgd[:], in_=agg[:])
                if li == 0:
                    xsl = sm_p.tile([F, W], f32, tag="xsl")
                    nc.sync.dma_start(xsl[:], T["xdt0"][:, ws])
                    xrhs = xsl[:]
                else:
                    xrhs = cur_d[:F, ws]
                out = ps_p.tile([D, W], f32, tag="out")
                nc.tensor.matmul(out[:], lhsT=wv64(f"wl_gd{li}", F),
                                 rhs=mgd[:], start=True, stop=False)
                nc.tensor.matmul(out[:], lhsT=wv64(f"wr_d{li}", F),
                                 rhs=xrhs, start=False, stop=True)
                ncols = cfg["last_d"] if w == nwin_d - 1 else W
                stats_cols(od, ws, w, wv(f"bd{li}")[:D, :], out, dsum, dsq,
                           ncols)
            bn_normalize(od, AD, dsum, dsq, nwin_d, cfg["ND"],
                         wv(f"gam_d{li}"), wv(f"bet_d{li}"),
                         cur_d if li > 0 else None)
            if li < 2:
                dsh = dram.tile([AD, D], f32, tag="dshard")
                transpose_out(od, 0, AD, dsh[:])
                nc.gpsimd.collective_compute(
                    "AllGather", mybir.AluOpType.bypass, replica_groups=rg,
                    ins=[dsh[:].opt()], outs=[xd_t[li + 1].opt()])
            cur_g, cur_d = og, od

        # ------------------------------------------------------------------
        # post MLP: h = lrelu(bn(x@W1+b1)); out = h@W2+b2
        hg, hd = gB, dB
        gsum = st_p.tile([D, nwin_g], f32, tag="gsum")
        gsq = st_p.tile([D, nwin_g], f32, tag="gsq")
        dsum = st_p.tile([D, nwin_d], f32, tag="dsum")
        dsq = st_p.tile([D, nwin_d], f32, tag="dsq")
        for w in range(nwin_g):
            ws = slice(w * W, (w + 1) * W)
            out = ps_p.tile([D, W], f32, tag="out")
            nc.tensor.matmul(out[:], lhsT=wv64("w1g"), rhs=cur_g[:D, ws],
                             start=True, stop=True)
            ncols = cfg["last_g"] if w == nwin_g - 1 else W
            stats_cols(hg, ws, w, wv("b1g")[:D, :], out, gsum, gsq, ncols)
        for w in range(nwin_d):
            ws = slice(w * W, (w + 1) * W)
            out = ps_p.tile([D, W], f32, tag="out")
            nc.tensor.matmul(out[:], lhsT=wv64("w1d"), rhs=cur_d[:D, ws],
                             start=True, stop=True)
            ncols = cfg["last_d"] if w == nwin_d - 1 else W
            stats_cols(hd, ws, w, wv("b1d")[:D, :], out, dsum, dsq, ncols)
        bn_normalize(hg, AG, gsum, gsq, nwin_g, cfg["NG"],
                     wv("gam_pg"), wv("bet_pg"), None)
        bn_normalize(hd, AD, dsum, dsq, nwin_d, cfg["ND"],
                     wv("gam_pd"), wv("bet_pd"), None)
        # lin2 into mean_g arena (gene rows 0:64; dis rows 64:128)
        for w in range(nwin_g):
            ws = slice(w * W, (w + 1) * W)
            out = ps_p.tile([D, W], f32, tag="out")
            nc.tensor.matmul(out[:], lhsT=wv64("w2g"), rhs=hg[:D, ws],
                             start=True, stop=True)
            nc.vector.tensor_scalar(out=mean_g[:D, ws], in0=out[:],
                                    scalar1=wv("b2g")[:D, :], scalar2=None,
                                    op0=OP.add)
        for w in range(nwin_d):
            ws = slice(w * W, (w + 1) * W)
            out = ps_p.tile([D, W], f32, tag="out")
            nc.tensor.matmul(out[:], lhsT=wv64("w2d"), rhs=hd[:D, ws],
                             start=True, stop=True)
            nc.vector.tensor_scalar(out=dA[:D, ws], in0=out[:],
                                    scalar1=wv("b2d")[:D, :], scalar2=None,
                                    op0=OP.add)
        transpose_out(mean_g, 0, AG, T["og_out"])
        transpose_out(dA, 0, AD, T["od_out"])


# ----------------------------------------------------------------------------
# Entry point
# ----------------------------------------------------------------------------

def kernel(x_gene, x_dis, e_gg, e_gd, e_dg, params):
    import concourse.bacc as bacc
    import concourse.tile as tile
    import concourse.mybir as mybir
    import concourse.bass_utils as bass_utils

    import time
    _t0 = time.time()

    def _log(msg):
        print(f"[kernel +{time.time()-_t0:7.1f}s] {msg}", flush=True)

    x_gene = np.asarray(x_gene)
    x_dis = np.asarray(x_dis)
    cfg, host = _prep(x_gene, x_dis, e_gg, e_gd, e_dg, params)
    _log(f"prep done sum_k gene={sum(sum(k) for k in cfg['gkws'])} "
         f"dis={sum(sum(k) for k in cfg['dkws'])}")

    f32 = mybir.dt.float32
    i32 = mybir.dt.int32
    nc = bacc.Bacc("TRN2", target_bir_lowering=False, debug=False,
                   enable_asserts=False, num_devices=C)

    T = {}
    T["xg0"] = nc.dram_tensor("xg0", list(host["xg0"].shape), f32,
                              kind="ExternalInput").ap()
    T["xd0"] = nc.dram_tensor("xd0", list(host["xd0"].shape), f32,
                              kind="ExternalInput").ap()
    T["xgt0"] = nc.dram_tensor("xgt0", list(host["xgt0"].shape[1:]), f32,
                               kind="ExternalInput").ap()
    T["xdt0"] = nc.dram_tensor("xdt0", list(host["xdt0"].shape[1:]), f32,
                               kind="ExternalInput").ap()
    T["gmeta"] = nc.dram_tensor("gmeta", list(host["gmeta"].shape[1:]), i32,
                                kind="ExternalInput").ap()
    T["dmeta"] = nc.dram_tensor("dmeta", list(host["dmeta"].shape[1:]), i32,
                                kind="ExternalInput").ap()
    T["wall"] = nc.dram_tensor("wall", list(host["wall"].shape), f32,
                               kind="ExternalInput").ap()
    T["og_out"] = nc.dram_tensor("og_out", [cfg["nwin_g"] * W, cfg["D"]], f32,
                                 kind="ExternalOutput").ap()
    T["od_out"] = nc.dram_tensor("od_out", [cfg["nwin_d"] * W, cfg["D"]], f32,
                                 kind="ExternalOutput").ap()
    dbg = os.environ.get("BASSGNN_DBG") == "1"
    if dbg:
        AG = cfg["nwin_g"] * W
        T["dbg_mean"] = nc.dram_tensor("dbg_mean", [P, AG], f32,
                                       kind="ExternalOutput").ap()
        T["dbg_og"] = nc.dram_tensor("dbg_og", [cfg["D"], AG], f32,
                                     kind="ExternalOutput").ap()
        T["dbg_sum"] = nc.dram_tensor("dbg_sum", [cfg["D"], cfg["nwin_g"]], f32,
                                      kind="ExternalOutput").ap()
        T["dbg_sq"] = nc.dram_tensor("dbg_sq", [cfg["D"], cfg["nwin_g"]], f32,
                                     kind="ExternalOutput").ap()
        T["dbg_rstat"] = nc.dram_tensor("dbg_rstat", [cfg["D"], 16], f32,
                                        kind="ExternalOutput").ap()
        T["dbg_var"] = nc.dram_tensor("dbg_var", [cfg["D"], 8], f32,
                                      kind="ExternalOutput").ap()

    with tile.TileContext(nc) as tc:
        _build(nc, tc, cfg, T)
    _log(f"trace+schedule done: {sum(len(b.instructions) for b in nc.main_func.blocks)} instructions")
    nc.compile()
    _log("bacc compile done")

    in_maps = []
    for c in range(C):
        in_maps.append({
            "xg0": host["xg0"], "xd0": host["xd0"],
            "xgt0": host["xgt0"][c], "xdt0": host["xdt0"][c],
            "gmeta": host["gmeta"][c], "dmeta": host["dmeta"][c],
            "wall": host["wall"],
        })

    if os.environ.get("BASSGNN_SIM") == "1":
        from concourse.bass_interp import MultiCoreSim
        sim = MultiCoreSim(nc, num_cores=C, trace=False,
                           require_finite=False, require_nnan=False)
        cores = list(sim.cores.values())
        for c, core in enumerate(cores):
            for k, v in in_maps[c].items():
                core.tensor(k)[:] = v
        try:
            sim.simulate(check_with_hw=False)
        except Exception as e:
            print("SIM EXCEPTION:", type(e).__name__, str(e)[:300])
        keys = ["og_out", "od_out"]
        if dbg:
            keys += ["dbg_mean", "dbg_og", "dbg_sum", "dbg_sq",
                     "dbg_rstat", "dbg_var"]
        results = [{k: np.array(core.tensor(k)) for k in keys}
                   for core in cores]
        kernel.debug_results = results
    else:
        _t1 = time.time()
        try:
            r = bass_utils.run_bass_kernel_spmd(
                nc, in_maps, core_ids=list(range(C)),
                trace=os.environ.get("BASSGNN_TRACE") == "1")
        except ModuleNotFoundError:
            r = bass_utils.run_bass_kernel_spmd(
                nc, in_maps, core_ids=list(range(C)), trace=False)
        _log(f"spmd run done (wall {time.time()-_t1:.1f}s, "
             f"exec_time_ns={r.exec_time_ns})")
        kernel.last_exec_time_ns = r.exec_time_ns
        results = r.results
        nrep = int(os.environ.get("BASSGNN_BENCH", "0"))
        if nrep:
            kernel.bench_ns = _bench(nc, in_maps, nrep)

    all_g = np.concatenate([results[c]["og_out"] for c in range(C)], axis=0)
    all_d = np.concatenate([results[c]["od_out"] for c in range(C)], axis=0)
    out_g = all_g[host["gpack"]].astype(np.float32)
    out_d = all_d[host["dpack"]].astype(np.float32)
    return out_g, out_d


def _bench(nc, in_maps, nrep):
    """Build the PJRT callable once, run nrep times, return min wall ns."""
    import time
    import jax
    import numpy as np
    from jax.sharding import Mesh, PartitionSpec
    from jax.experimental.shard_map import shard_map
    import concourse.bass2jax as b2j
    import concourse.mybir as mybir

    b2j.install_neuronx_cc_hook()
    partition_name = (nc.partition_id_tensor.name
                      if nc.partition_id_tensor else None)
    in_names, out_names, out_avals, zero_outs = [], [], [], []
    for alloc in nc.m.functions[0].allocations:
        if not isinstance(alloc, mybir.MemoryLocationSet):
            continue
        name = alloc.memorylocations[0].name
        if alloc.kind == "ExternalInput":
            if name != partition_name:
                in_names.append(name)
        elif alloc.kind == "ExternalOutput":
            out_names.append(name)
            shape = tuple(alloc.tensor_shape)
            dtype = mybir.dt.np(alloc.dtype)
            out_avals.append(jax.core.ShapedArray(shape, dtype))
            zero_outs.append(np.zeros(shape, dtype))
    n_params = len(in_names)
    n_outs = len(out_avals)
    in_names = in_names + out_names
    if partition_name is not None:
        in_names.append(partition_name)
    donate = tuple(range(n_params, n_params + n_outs))

    def _body(*args):
        operands = list(args)
        if partition_name is not None:
            operands.append(b2j.partition_id_tensor())
        outs = b2j._bass_exec_p.bind(
            *operands, out_avals=tuple(out_avals), in_names=tuple(in_names),
            out_names=tuple(out_names), lowering_input_output_aliases=(),
            sim_require_finite=True, sim_require_nnan=True, nc=nc)
        return tuple(outs)

    devices = jax.devices()[:C]
    mesh = Mesh(np.asarray(devices), ("core",))
    sharded = jax.jit(
        shard_map(_body, mesh=mesh,
                  in_specs=(PartitionSpec("core"),) * (n_params + n_outs),
                  out_specs=(PartitionSpec("core"),) * n_outs,
                  check_rep=False),
        keep_unused=True)
    concat_in = [
        np.concatenate([np.asarray(in_maps[c][n]) for c in range(C)], axis=0)
        for n in in_names[:n_params]]
    concat_zeros = [np.zeros((C * z.shape[0], *z.shape[1:]), z.dtype)
                    for z in zero_outs]
    args_dev = jax.device_put(concat_in + concat_zeros)
    out = sharded(*args_dev)
    jax.block_until_ready(out)
    times = []
    for _ in range(nrep):
        t0 = time.perf_counter()
        out = sharded(*args_dev)
        jax.block_until_ready(out)
        times.append(time.perf_counter() - t0)
    times = np.array(times) * 1e9
    print(f"[bench] n={nrep} min={times.min()/1e6:.3f}ms "
          f"p50={np.percentile(times,50)/1e6:.3f}ms "
          f"mean={times.mean()/1e6:.3f}ms", flush=True)
    return float(times.min())


kernel.last_exec_time_ns = None
